# revision 1
# baseline (speedup 1.0000x reference)
"""Self-attention (8 heads, d=64, B=2, N=4096, D=512) on 8 TRN2 NeuronCores.

Sharding: batch*heads across cores — core c handles batch b=c//4, heads
(2*(c%4), 2*(c%4)+1). Projection weights are sliced per-core on the host;
x is pre-transposed on the host so the device needs no transposes at all.

Device dataflow (per core, fully transposed "scoresT" formulation):
  qT2/kT2 [hd=128, n]  = W.T-chunks @ xT-chunks          (PE, f32r)
  v2      [n, hd+ones] natural                            (PE, bf16 store)
  per head h, per q-chunk qq (1024 wide):
    for kc in 32:  scT psum[128k,1024q] = kh.T @ qh       (PE)
                   attnT = exp(scT*SCALE)  -> bf16 SBUF   (ACT, scale fused)
                   av[65,1024] += v2'[kc].T @ attnT       (PE, accumulate)
    row 64 of av = softmax denominator (ones column of v2')
    outT[h] = av[:64] * (1/denom)                         (DVE + DMA bcast)
  partial[n,512] = sum_h outT[h].T @ woT[h]               (PE)
Host: out[b] = sum of its 4 cores' partials + bo.
"""
import numpy as np
import ml_dtypes
from contextlib import ExitStack

import concourse.bass as bass
from concourse import bacc
import concourse.mybir as mybir
import concourse.tile as tile
from concourse.bass_utils import run_bass_kernel_spmd

B, N, D = 2, 4096, 512
HEADS, DH = 8, 64
SCALE = DH ** -0.5

F32 = mybir.dt.float32
F32R = mybir.dt.bfloat16  # matmul operand dtype (bf16: 1cyc/row, standard path)
BF16 = mybir.dt.bfloat16

QQ_W = 1024          # q-chunk width in the attention loop
N_QQ = N // QQ_W     # 4
N_KC = N // 128      # 32 key chunks
DCH = D // 128       # 4 contraction chunks for projections


def build_bass():
    nc = bacc.Bacc(None, target_bir_lowering=False)

    xT = nc.dram_tensor("xT", [D, N], F32R, kind="ExternalInput")
    wqT = nc.dram_tensor("wqT", [D, 128], F32R, kind="ExternalInput")
    wkT = nc.dram_tensor("wkT", [D, 128], F32R, kind="ExternalInput")
    wvT = nc.dram_tensor("wvT", [D, 128], F32R, kind="ExternalInput")
    woT = nc.dram_tensor("woT", [2, 64, D], F32R, kind="ExternalInput")
    out = nc.dram_tensor("out", [N, D], F32, kind="ExternalOutput")
    recip_dram = nc.dram_tensor("recip_scratch", [N_QQ, 2, QQ_W], F32)

    with tile.TileContext(nc) as tc, ExitStack() as ctx:
        const = ctx.enter_context(tc.tile_pool(name="const", bufs=1))

        # ---- load inputs ----
        xT_sb = const.tile([128, DCH, N], F32R)            # xT[(c p), n] -> [p, c, n]
        nc.sync.dma_start(out=xT_sb, in_=xT.rearrange("(c p) n -> p c n", p=128))
        wq_sb = const.tile([128, DCH, 128], F32R)
        nc.sync.dma_start(out=wq_sb, in_=wqT.rearrange("(c p) m -> p c m", p=128))
        wk_sb = const.tile([128, DCH, 128], F32R)
        nc.sync.dma_start(out=wk_sb, in_=wkT.rearrange("(c p) m -> p c m", p=128))
        wv_sb = const.tile([128, DCH, 128], F32R)
        nc.sync.dma_start(out=wv_sb, in_=wvT.rearrange("(c p) m -> p c m", p=128))
        wo_sb = const.tile([64, 2, D], F32R)
        nc.sync.dma_start(out=wo_sb, in_=woT.rearrange("h d n -> d h n"))

        qT2 = const.tile([128, N], F32R)                   # [2-head d, n]
        kT2 = const.tile([128, N], F32R)
        v2 = const.tile([128, N_KC, 130], BF16)            # [k-part, kc, (v_h0|1|v_h1|1)]
        outT = const.tile([64, 2, N], F32R)                # normalized per-head av

        # ---- projections ----
        with tc.tile_pool(name="proj_psum", bufs=3, space="PSUM") as proj_psum:
            for nt in range(N // 512):
                pq = proj_psum.tile([128, 512], F32, tag="pj")
                for c in range(DCH):
                    nc.tensor.matmul(pq, wq_sb[:, c, :], xT_sb[:, c, bass.ts(nt, 512)],
                                     start=(c == 0), stop=(c == DCH - 1))
                nc.vector.tensor_copy(qT2[:, bass.ts(nt, 512)], pq)
            for nt in range(N // 512):
                pk = proj_psum.tile([128, 512], F32, tag="pj")
                for c in range(DCH):
                    nc.tensor.matmul(pk, wk_sb[:, c, :], xT_sb[:, c, bass.ts(nt, 512)],
                                     start=(c == 0), stop=(c == DCH - 1))
                nc.vector.tensor_copy(kT2[:, bass.ts(nt, 512)], pk)
            # v natural: out[n-tile, hd] = xT-chunk.T @ wv-chunk
            for kc in range(N_KC):
                pv = proj_psum.tile([128, 128], F32, tag="pv")
                for c in range(DCH):
                    nc.tensor.matmul(pv, xT_sb[:, c, bass.ts(kc, 128)], wv_sb[:, c, :],
                                     start=(c == 0), stop=(c == DCH - 1))
                # interleave the two heads' 64-col halves into v2 (cols 0-63, 65-128)
                nc.vector.tensor_copy(v2[:, kc, 0:64], pv[:, 0:64])
                nc.vector.tensor_copy(v2[:, kc, 65:129], pv[:, 64:128])
        # ones columns for the softmax-denominator trick
        nc.vector.memset(v2[:, :, 64], 1.0)
        nc.vector.memset(v2[:, :, 129], 1.0)

        # ---- attention ----
        with (
            tc.tile_pool(name="sc_psum", bufs=2, space="PSUM") as sc_psum,
            tc.tile_pool(name="av_psum", bufs=2, space="PSUM") as av_psum,
            tc.tile_pool(name="attn_sb", bufs=4) as attn_sb,
            tc.tile_pool(name="norm_sb", bufs=2) as norm_sb,
        ):
            for qq in range(N_QQ):
                avs = []
                for h in range(2):
                    av = av_psum.tile([65, QQ_W], F32, tag="av", name=f"av_{qq}_{h}")
                    avs.append(av)
                for kc in range(N_KC):
                    for h in range(2):
                        sc = sc_psum.tile([128, QQ_W], F32, tag="sc", name=f"sc_{qq}_{kc}_{h}")
                        for s in range(QQ_W // 512):
                            nc.tensor.matmul(
                                sc[:, bass.ts(s, 512)],
                                kT2[h * 64:(h + 1) * 64, bass.ts(kc, 128)],
                                qT2[h * 64:(h + 1) * 64, qq * QQ_W + s * 512:qq * QQ_W + (s + 1) * 512],
                                start=True, stop=True)
                        at = attn_sb.tile([128, QQ_W], BF16, tag="at", name=f"at_{qq}_{kc}_{h}")
                        nc.scalar.activation(at, sc, mybir.ActivationFunctionType.Exp,
                                             scale=float(SCALE))
                        for s in range(QQ_W // 512):
                            nc.tensor.matmul(
                                avs[h][:, bass.ts(s, 512)],
                                v2[:, kc, h * 65:(h + 1) * 65],
                                at[:, bass.ts(s, 512)],
                                start=(kc == 0), stop=(kc == N_KC - 1))
                # normalize: outT[h][:, qq] = av[:64] * 1/av[64]
                for h in range(2):
                    av = avs[h]
                    rc = norm_sb.tile([128, QQ_W], F32, tag="rc", name=f"rc_{qq}_{h}")
                    nc.vector.reciprocal(rc[64:65, :], av[64:65, :])
                    bc = norm_sb.tile([64, QQ_W], F32, tag="bc", name=f"bc_{qq}_{h}")
                    nc.sync.dma_start(out=recip_dram[qq:qq + 1, h, :], in_=rc[64:65, :])
                    src = recip_dram[qq, h, :]
                    bcast = bass.AP(tensor=src.tensor, offset=src.offset,
                                    ap=[[0, 64]] + src.ap)
                    nc.sync.dma_start(out=bc, in_=bcast)
                    nc.vector.tensor_mul(outT[:, h, qq * QQ_W:(qq + 1) * QQ_W], av[0:64, :], bc)

        # ---- output projection ----
        with (
            tc.tile_pool(name="op_psum", bufs=3, space="PSUM") as op_psum,
            tc.tile_pool(name="op_sb", bufs=3) as op_sb,
        ):
            for nt in range(N // 128):
                po = op_psum.tile([128, D], F32, tag="po")
                nc.tensor.matmul(po, outT[:, 0, bass.ts(nt, 128)], wo_sb[:, 0, :],
                                 start=True, stop=False)
                nc.tensor.matmul(po, outT[:, 1, bass.ts(nt, 128)], wo_sb[:, 1, :],
                                 start=False, stop=True)
                ob = op_sb.tile([128, D], F32, tag="ob")
                nc.vector.tensor_copy(ob, po)
                nc.sync.dma_start(out=out[bass.ts(nt, 128), :], in_=ob)

    nc.compile()
    return nc


_NC_CACHE = None


def build_in_maps(x, Wq, Wk, Wv, Wo):
    bf = ml_dtypes.bfloat16
    x = np.asarray(x, np.float32)
    Wq, Wk, Wv, Wo = (np.asarray(a, np.float32) for a in (Wq, Wk, Wv, Wo))
    in_maps = []
    for c in range(8):
        b = c // 4
        h0 = 2 * (c % 4)
        xT = np.ascontiguousarray(x[b].T.astype(bf))
        wqT = np.ascontiguousarray(Wq[h0 * 64:(h0 + 2) * 64].T.astype(bf))
        wkT = np.ascontiguousarray(Wk[h0 * 64:(h0 + 2) * 64].T.astype(bf))
        wvT = np.ascontiguousarray(Wv[h0 * 64:(h0 + 2) * 64].T.astype(bf))
        woT = np.stack([np.ascontiguousarray(Wo[:, (h0 + h) * 64:(h0 + h + 1) * 64].T.astype(bf))
                        for h in range(2)])
        in_maps.append({"xT": xT, "wqT": wqT, "wkT": wkT, "wvT": wvT, "woT": woT})
    return in_maps


def kernel(x, Wq, Wk, Wv, Wo, bo):
    global _NC_CACHE
    bo = np.asarray(bo, np.float32)
    in_maps = build_in_maps(x, Wq, Wk, Wv, Wo)

    if _NC_CACHE is None:
        _NC_CACHE = build_bass()
    res = run_bass_kernel_spmd(_NC_CACHE, in_maps, list(range(8)))
    partials = [np.asarray(res.results[c]["out"], np.float32) for c in range(8)]

    out = np.empty((B, N, D), np.float32)
    for b in range(B):
        out[b] = partials[4 * b] + partials[4 * b + 1] + partials[4 * b + 2] + partials[4 * b + 3] + bo
    return out


if __name__ == "__main__":
    nc = build_bass()
    print("built ok")

